# revision 5
# baseline (speedup 1.0000x reference)
"""Trainium2 Bass kernel for ExpertBranch: fp8-blockwise-fakequant FFN.

  h   = gelu_tanh(fq8(x) @ fq8_rows(kernel1) + bias1)
  out = fq8(h) @ fq8_rows(kernel2) + bias2

Sharding: data-parallel over the 8192 flattened rows of x - each of the 8
NeuronCores computes a 1024-row slice with replicated weights. No collectives.

Structure (per core, phases overlap via the Tile scheduler):
  A: x blockwise-fp8 fake-quant (RNE via Act-engine halved-scale TRN-e4m3
     trick) + PE transpose -> xT resident in SBUF (fp16), 32 quarter-tiles.
  B: GEMM1 (fp16, 256-wide n-tiles, PSUM k-accum) + bias1 + native
     Gelu_apprx_tanh on the scalar engine + h fake-quant + PE transpose
     -> hT fully resident in SBUF (fp16, 128 KiB/part; no DRAM staging).
     Split in two m-halves so the first half overlaps phase A.
  C: GEMM2 (fp16 x fp16) streaming w2 once in 4-k-block chunks + bias2;
     psum drains interleaved into the final chunk's matmul stream; the w2
     pool lives in virgin SBUF so chunk prefetch starts during B.

Engine budget per B-tile: PE 1.81us (16 matmuls + 2 transposes), Act 1.46us
(gelu + 2 RNE fp8 quants + 2 dequants), DVE ~1.1us (bias-add, amax reduce,
hT copy), gpsimd: scale smalls. fp16 operand paths (not bf16): same PE
speed, 8x less rounding noise - the h fake-quant amplifies pre-quant noise
eps to sqrt(eps*bin), so bf16 costs ~1.2e-2 rel err vs fp16 ~4.7e-3.

Weights are fake-quantized on the host (numpy, bitwise-exact OCP e4m3fn
semantics) - static preprocessing; all activation work (x-quant, GEMMs,
gelu, h-quant) runs on device. TimelineSim: ~2.294 ms/core (baseline 2.643).
"""

import contextlib
import sys

import numpy as np

sys.path.insert(0, "/opt/trn_rl_repo")

import ml_dtypes  # noqa: E402

import concourse.bacc as bacc  # noqa: E402
import concourse.bass as bass  # noqa: E402
import concourse.mybir as mybir  # noqa: E402
import concourse.tile as tile  # noqa: E402
from concourse.masks import make_identity  # noqa: E402
from concourse.bass_utils import run_bass_kernel_spmd  # noqa: E402

F32 = mybir.dt.float32
F16 = mybir.dt.float16
FP8 = mybir.dt.float8e4

P = 128          # partitions
NCORES = 8
D_MODEL = 2048
EXPERT = 8192
ROWS = 4 * 2048  # flattened x rows
MC = ROWS // NCORES   # rows per core = 1024
MT = MC // P          # m-tiles per core = 8
KB1 = D_MODEL // P    # k-blocks GEMM1 = 16
NB = 256              # GEMM1 n-tile width
NT1 = EXPERT // NB    # n-tiles GEMM1 = 32
KB2 = EXPERT // P     # k-blocks GEMM2 = 64
KC = 4                # k-blocks per w2 stream chunk
NKC = KB2 // KC       # chunks = 16
JT = EXPERT // 512    # j-tiles GEMM2 = 16
J = 512

C224INV = float(np.float32(1.0 / 224.0))
EPS = 1e-12
GELU = mybir.ActivationFunctionType.Gelu_apprx_tanh
COPY = mybir.ActivationFunctionType.Copy
ADD = mybir.AluOpType.add
MULT = mybir.AluOpType.mult
MAXOP = mybir.AluOpType.max


def _build():
    nc = bacc.Bacc("TRN2", target_bir_lowering=False, debug=False)

    x_in = nc.dram_tensor("xp", [P, MT, D_MODEL], F32, kind="ExternalInput")
    w1_in = nc.dram_tensor("w1p", [P, KB1, EXPERT], F16, kind="ExternalInput")
    b1_in = nc.dram_tensor("b1", [EXPERT], F32, kind="ExternalInput")
    w2_in = nc.dram_tensor("w2p", [P, KB2, EXPERT], F16, kind="ExternalInput")
    b2_in = nc.dram_tensor("b2", [EXPERT], F32, kind="ExternalInput")
    out = nc.dram_tensor("out", [MC, EXPERT], F32, kind="ExternalOutput")

    with tile.TileContext(nc) as tc, contextlib.ExitStack() as top:
        const = top.enter_context(tc.tile_pool(name="const", bufs=1))
        ident_f = const.tile([P, P], F32)
        make_identity(nc, ident_f[:])
        ident = const.tile([P, P], F16)
        nc.vector.tensor_copy(ident[:], ident_f[:])
        warm = const.tile([P, 1], F32)
        nc.scalar.activation(warm[:], ident_f[:, 0:1], GELU)

        # Resident activations (SBUF): quantized-transposed x and h.
        xT_pool = top.enter_context(tc.tile_pool(name="xT", bufs=1))
        xT = xT_pool.tile([P, KB1, MC], F16)   # 32 KiB/part
        hT_pool = top.enter_context(tc.tile_pool(name="hT", bufs=1))
        hT = hT_pool.tile([P, KB2, MC], F16)   # 128 KiB/part

        # w2 stream pool opened first (virgin SBUF) so phase C's first chunk
        # prefetches during phase B without WAR stalls on freed space.
        w2p = top.enter_context(tc.tile_pool(name="w2p", bufs=2))

        # --- B-phase pools (virgin SBUF; opened before A so B never WARs
        #     on A space) ---
        b_stack = contextlib.ExitStack()
        w1p = b_stack.enter_context(tc.tile_pool(name="w1p", bufs=2))
        b1p = b_stack.enter_context(tc.tile_pool(name="b1p", bufs=3))
        zp = b_stack.enter_context(tc.tile_pool(name="zp", bufs=4))
        gp = b_stack.enter_context(tc.tile_pool(name="gp", bufs=4))
        h8p = b_stack.enter_context(tc.tile_pool(name="h8p", bufs=4))
        hqp = b_stack.enter_context(tc.tile_pool(name="hqp", bufs=4))
        scb = b_stack.enter_context(tc.tile_pool(name="scb", bufs=6))
        pp = b_stack.enter_context(tc.tile_pool(name="pp", bufs=4, space="PSUM"))
        ptb = b_stack.enter_context(tc.tile_pool(name="ptb", bufs=2, space="PSUM"))

        # --- A-phase pools ---
        a_stack = contextlib.ExitStack()
        xa = a_stack.enter_context(tc.tile_pool(name="xa", bufs=2))
        q8a = a_stack.enter_context(tc.tile_pool(name="q8a", bufs=2))
        xqa = a_stack.enter_context(tc.tile_pool(name="xqa", bufs=2))
        sca = a_stack.enter_context(tc.tile_pool(name="sca", bufs=4))
        pta = a_stack.enter_context(tc.tile_pool(name="pta", bufs=2, space="PSUM"))

        # ---------------- Phase A: x quant + transpose -> xT (bf16) --------
        # processed in 32 quarter-row-tiles [P, 512] (4 k-blocks each)
        HB = 4                      # k-blocks per quarter-tile
        HN = HB * P                 # 512 columns
        for mi in range(MT):
            for h in range(4):
                xt = xa.tile([P, HN], F32)
                nc.sync.dma_start(out=xt[:], in_=x_in[:, mi, HN * h:HN * (h + 1)])
                amax = sca.tile([P, HB], F32, tag="amax")
                nc.vector.tensor_reduce(
                    amax[:], xt[:].rearrange("p (kb b) -> p kb b", b=P),
                    axis=mybir.AxisListType.X, op=MAXOP,
                    apply_absolute_value=True)
                nc.gpsimd.tensor_scalar_max(amax[:], amax[:], EPS)
                rcp = sca.tile([P, HB], F32, tag="rcp")
                nc.vector.reciprocal_approx_fast(out=rcp[:], in_=amax[:])
                inv2 = sca.tile([P, HB], F32, tag="inv2")
                nc.gpsimd.tensor_scalar_mul(inv2[:], rcp[:], 224.0)
                s2 = sca.tile([P, HB], F32, tag="s2")
                nc.gpsimd.tensor_scalar_mul(s2[:], amax[:], C224INV)
                q8 = q8a.tile([P, HN], FP8)
                xq = xqa.tile([P, HN], F16)
                for b in range(HB):
                    sl = slice(P * b, P * (b + 1))
                    # fp8 code: RNE(x * (224/amax)) via halved-scale TRN-e4m3
                    nc.scalar.activation(q8[:, sl], xt[:, sl], COPY,
                                         scale=inv2[:, b:b + 1])
                    # dequant: code * (amax/224) -> bf16 (split DVE/Act)
                    if b < 3:
                        nc.vector.tensor_scalar(
                            xq[:, sl], q8[:, sl], s2[:, b:b + 1], None,
                            op0=MULT)
                    else:
                        nc.scalar.activation(xq[:, sl], q8[:, sl], COPY,
                                             scale=s2[:, b:b + 1])
                pt = pta.tile([P, HB, P], F16)
                for b in range(HB):
                    nc.tensor.transpose(pt[:, b, :], xq[:, P * b:P * (b + 1)],
                                        ident[:])
                nc.vector.tensor_copy(
                    xT[:, HB * h:HB * (h + 1), P * mi:P * (mi + 1)], pt[:])
        a_stack.close()

        # ------- Phase B: GEMM1 + bias + gelu + h-quant + transpose -------
        # n-tiles 256 wide; m-halves so B(q=0) starts once A(mi<4) is done
        for q in range(2):
            for ni in range(NT1):
                w1t = w1p.tile([P, KB1, NB], F16)
                nc.sync.dma_start(
                    out=w1t[:], in_=w1_in[:, :, NB * ni:NB * (ni + 1)])
                b1t = b1p.tile([P, NB], F32)
                nc.sync.dma_start(
                    out=b1t[:], in_=bass.AP(b1_in, NB * ni, [[0, P], [1, NB]]))
                for mi in range(4 * q, 4 * q + 4):
                    ps = pp.tile([P, NB], F32)
                    for kb in range(KB1):
                        nc.tensor.matmul(
                            ps[:], xT[:, kb, P * mi:P * (mi + 1)], w1t[:, kb, :],
                            start=(kb == 0), stop=(kb == KB1 - 1))
                    z = zp.tile([P, NB], F32)
                    nc.vector.tensor_tensor(z[:], ps[:], b1t[:], op=ADD)
                    g = gp.tile([P, NB], F32)
                    nc.scalar.activation(g[:], z[:], GELU)
                    amaxh = scb.tile([P, 2], F32, tag="amaxh")
                    nc.vector.tensor_reduce(
                        amaxh[:], g[:].rearrange("p (nb b) -> p nb b", b=P),
                        axis=mybir.AxisListType.X, op=MAXOP,
                        apply_absolute_value=True)
                    nc.vector.tensor_scalar_max(amaxh[:], amaxh[:], EPS)
                    rch = scb.tile([P, 2], F32, tag="rch")
                    nc.vector.reciprocal_approx_fast(out=rch[:], in_=amaxh[:])
                    inv2h = scb.tile([P, 2], F32, tag="inv2h")
                    nc.vector.tensor_scalar_mul(inv2h[:], rch[:], 224.0)
                    s2h = scb.tile([P, 2], F32, tag="s2h")
                    nc.vector.tensor_scalar_mul(s2h[:], amaxh[:], C224INV)
                    h8 = h8p.tile([P, NB], FP8)
                    hq = hqp.tile([P, NB], F16)
                    for b in range(2):
                        sl = slice(P * b, P * (b + 1))
                        nc.scalar.activation(h8[:, sl], g[:, sl], COPY,
                                             scale=inv2h[:, b:b + 1])
                        nc.scalar.activation(hq[:, sl], h8[:, sl], COPY,
                                             scale=s2h[:, b:b + 1])
                    if mi % 2 == 0:
                        pt = ptb.tile([P, 2, 2, P], F16, tag="pt")
                        pt_pair = pt
                    for b in range(2):
                        nc.tensor.transpose(pt_pair[:, b, mi % 2, :],
                                            hq[:, P * b:P * (b + 1)], ident[:])
                    if mi % 2 == 1:
                        nc.vector.tensor_copy(
                            hT[:, 2 * ni:2 * ni + 2, P * (mi - 1):P * (mi + 1)],
                            pt_pair[:])
        b_stack.close()

        # ---------------- Phase C: GEMM2 + bias2 ----------------
        with contextlib.ExitStack() as c_stack:
            b2p = c_stack.enter_context(tc.tile_pool(name="b2p", bufs=2))
            op_ = c_stack.enter_context(tc.tile_pool(name="op", bufs=4))
            pc = c_stack.enter_context(tc.tile_pool(name="pc", bufs=8,
                                                    space="PSUM"))
            for ji in range(JT):
                b2t = b2p.tile([P, J], F32)
                nc.sync.dma_start(
                    out=b2t[:], in_=bass.AP(b2_in, J * ji, [[0, P], [1, J]]))
                pss = [pc.tile([P, J], F32, name="pss", tag="pss")
                       for _ in range(MT)]
                for kc in range(NKC):
                    w2c = w2p.tile([P, KC, J], F16)
                    nc.sync.dma_start(
                        out=w2c[:],
                        in_=w2_in[:, KC * kc:KC * (kc + 1), J * ji:J * (ji + 1)])
                    for mi in range(MT):
                        for kb in range(KC):
                            nc.tensor.matmul(
                                pss[mi][:],
                                hT[:, KC * kc + kb, P * mi:P * (mi + 1)],
                                w2c[:, kb, :],
                                start=(kc == 0 and kb == 0),
                                stop=(kc == NKC - 1 and kb == KC - 1))
                        if kc == NKC - 1:
                            # drain interleaved with remaining mi matmuls
                            ot = op_.tile([P, J], F32)
                            nc.vector.tensor_tensor(
                                ot[:], pss[mi][:], b2t[:], op=ADD)
                            dmaeng = nc.scalar if mi % 2 else nc.sync
                            dmaeng.dma_start(
                                out=out[P * mi:P * (mi + 1),
                                        J * ji:J * (ji + 1)],
                                in_=ot[:])

    nc.compile()
    return nc


_NC = None
last_results = None


def _get_nc():
    global _NC
    if _NC is None:
        _NC = _build()
    return _NC


def _fq8_rows(w: np.ndarray) -> np.ndarray:
    """Reference fp8 row-blockwise fake-quant (bitwise-exact, OCP e4m3fn)."""
    K, N = w.shape
    wb = w.reshape(K // P, P, N)
    scale = (np.maximum(np.abs(wb).max(axis=1, keepdims=True), EPS)
             / np.float32(448.0)).astype(np.float32)
    q = (wb / scale).astype(ml_dtypes.float8_e4m3fn).astype(np.float32) * scale
    return q.reshape(K, N).astype(np.float32)


def _prepare_in_maps(x, kernel1, bias1, kernel2, bias2):
    x = np.ascontiguousarray(np.asarray(x, dtype=np.float32))
    k1 = np.asarray(kernel1, dtype=np.float32)
    k2 = np.asarray(kernel2, dtype=np.float32)
    b1 = np.ascontiguousarray(np.asarray(bias1, dtype=np.float32))
    b2 = np.ascontiguousarray(np.asarray(bias2, dtype=np.float32))

    # Host-side static weight fake-quant (+ packing to [P, K//P, N] bf16).
    w1q = _fq8_rows(k1)
    w2q = _fq8_rows(k2)
    w1p = np.ascontiguousarray(
        w1q.reshape(KB1, P, EXPERT).transpose(1, 0, 2).astype(np.float16))
    w2p = np.ascontiguousarray(
        w2q.reshape(KB2, P, EXPERT).transpose(1, 0, 2).astype(np.float16))

    xf = x.reshape(ROWS, D_MODEL)
    in_maps = []
    for c in range(NCORES):
        xs = xf[MC * c:MC * (c + 1)]
        xp = np.ascontiguousarray(xs.reshape(MT, P, D_MODEL).transpose(1, 0, 2))
        in_maps.append({"xp": xp, "w1p": w1p, "b1": b1, "w2p": w2p, "b2": b2})
    return in_maps


def kernel(x, kernel1, bias1, kernel2, bias2):
    global last_results
    nc = _get_nc()
    in_maps = _prepare_in_maps(x, kernel1, bias1, kernel2, bias2)
    last_results = run_bass_kernel_spmd(nc, in_maps, core_ids=list(range(NCORES)))
    outs = [last_results.results[c]["out"] for c in range(NCORES)]
    full = np.concatenate(outs, axis=0).reshape(4, 2048, EXPERT)
    return full.astype(np.float32)


# revision 6
# speedup vs baseline: 1.0036x; 1.0036x over previous
"""Trainium2 Bass kernel for ExpertBranch: fp8-blockwise-fakequant FFN.

  h   = gelu_tanh(fq8(x) @ fq8_rows(kernel1) + bias1)
  out = fq8(h) @ fq8_rows(kernel2) + bias2

Sharding: data-parallel over the 8192 flattened rows of x - each of the 8
NeuronCores computes a 1024-row slice with replicated weights. No collectives.

Structure (per core, phases overlap via the Tile scheduler):
  A: x blockwise-fp8 fake-quant (RNE via Act-engine halved-scale TRN-e4m3
     trick) + PE transpose -> xT resident in SBUF (fp16), 32 quarter-tiles.
  B: GEMM1 (fp16, 256-wide n-tiles, PSUM k-accum) + bias1 + native
     Gelu_apprx_tanh on the scalar engine + h fake-quant + PE transpose
     -> hT fully resident in SBUF (fp16, 128 KiB/part; no DRAM staging).
     Split in two m-halves so the first half overlaps phase A.
  C: GEMM2 (fp16 x fp16) streaming w2 once in 4-k-block chunks + bias2;
     psum drains interleaved into the final chunk's matmul stream; the w2
     pool lives in virgin SBUF so chunk prefetch starts during B.

Engine budget per B-tile: PE 1.81us (16 matmuls + 2 transposes), Act 1.46us
(gelu + 2 RNE fp8 quants + 2 dequants), DVE ~1.1us (bias-add, amax reduce,
hT copy), gpsimd: scale smalls. fp16 operand paths (not bf16): same PE
speed, 8x less rounding noise - the h fake-quant amplifies pre-quant noise
eps to sqrt(eps*bin), so bf16 costs ~1.2e-2 rel err vs fp16 ~4.7e-3.

Weights are fake-quantized on the host (numpy, bitwise-exact OCP e4m3fn
semantics) - static preprocessing; all activation work (x-quant, GEMMs,
gelu, h-quant) runs on device. TimelineSim: ~2.294 ms/core (baseline 2.643).
"""

import contextlib
import sys

import numpy as np

sys.path.insert(0, "/opt/trn_rl_repo")

import ml_dtypes  # noqa: E402

import concourse.bacc as bacc  # noqa: E402
import concourse.bass as bass  # noqa: E402
import concourse.mybir as mybir  # noqa: E402
import concourse.tile as tile  # noqa: E402
from concourse.masks import make_identity  # noqa: E402
from concourse.bass_utils import run_bass_kernel_spmd  # noqa: E402

F32 = mybir.dt.float32
F16 = mybir.dt.float16
FP8 = mybir.dt.float8e4

P = 128          # partitions
NCORES = 8
D_MODEL = 2048
EXPERT = 8192
ROWS = 4 * 2048  # flattened x rows
MC = ROWS // NCORES   # rows per core = 1024
MT = MC // P          # m-tiles per core = 8
KB1 = D_MODEL // P    # k-blocks GEMM1 = 16
NB = 256              # GEMM1 n-tile width
NT1 = EXPERT // NB    # n-tiles GEMM1 = 32
KB2 = EXPERT // P     # k-blocks GEMM2 = 64
KC = 4                # k-blocks per w2 stream chunk
NKC = KB2 // KC       # chunks = 16
JT = EXPERT // 512    # j-tiles GEMM2 = 16
J = 512

C224INV = float(np.float32(1.0 / 224.0))
EPS = 1e-12
GELU = mybir.ActivationFunctionType.Gelu_apprx_tanh
COPY = mybir.ActivationFunctionType.Copy
ADD = mybir.AluOpType.add
MULT = mybir.AluOpType.mult
MAXOP = mybir.AluOpType.max


def _build():
    nc = bacc.Bacc("TRN2", target_bir_lowering=False, debug=False)

    x_in = nc.dram_tensor("xp", [P, MT, D_MODEL], F32, kind="ExternalInput")
    w1_in = nc.dram_tensor("w1p", [P, KB1, EXPERT], F16, kind="ExternalInput")
    b1_in = nc.dram_tensor("b1", [EXPERT], F32, kind="ExternalInput")
    w2_in = nc.dram_tensor("w2p", [P, KB2, EXPERT], F16, kind="ExternalInput")
    b2_in = nc.dram_tensor("b2", [EXPERT], F32, kind="ExternalInput")
    out = nc.dram_tensor("out", [MC, EXPERT], F32, kind="ExternalOutput")

    with tile.TileContext(nc) as tc, contextlib.ExitStack() as top:
        const = top.enter_context(tc.tile_pool(name="const", bufs=1))
        ident_f = const.tile([P, P], F32)
        make_identity(nc, ident_f[:])
        ident = const.tile([P, P], F16)
        nc.vector.tensor_copy(ident[:], ident_f[:])
        warm = const.tile([P, 1], F32)
        nc.scalar.activation(warm[:], ident_f[:, 0:1], GELU)

        # Resident activations (SBUF): quantized-transposed x and h.
        xT_pool = top.enter_context(tc.tile_pool(name="xT", bufs=1))
        xT = xT_pool.tile([P, KB1, MC], F16)   # 32 KiB/part
        hT_pool = top.enter_context(tc.tile_pool(name="hT", bufs=1))
        hT = hT_pool.tile([P, KB2, MC], F16)   # 128 KiB/part

        # w2 stream pool opened first (virgin SBUF) so phase C's first chunk
        # prefetches during phase B without WAR stalls on freed space.
        w2p = top.enter_context(tc.tile_pool(name="w2p", bufs=2))

        # --- B-phase pools (virgin SBUF; opened before A so B never WARs
        #     on A space) ---
        b_stack = contextlib.ExitStack()
        w1p = b_stack.enter_context(tc.tile_pool(name="w1p", bufs=2))
        b1p = b_stack.enter_context(tc.tile_pool(name="b1p", bufs=2))
        zp = b_stack.enter_context(tc.tile_pool(name="zp", bufs=4))
        gp = b_stack.enter_context(tc.tile_pool(name="gp", bufs=4))
        h8p = b_stack.enter_context(tc.tile_pool(name="h8p", bufs=4))
        hqp = b_stack.enter_context(tc.tile_pool(name="hqp", bufs=4))
        scb = b_stack.enter_context(tc.tile_pool(name="scb", bufs=4))
        pp = b_stack.enter_context(tc.tile_pool(name="pp", bufs=4, space="PSUM"))
        ptb = b_stack.enter_context(tc.tile_pool(name="ptb", bufs=2, space="PSUM"))

        # --- A-phase pools ---
        a_stack = contextlib.ExitStack()
        xa = a_stack.enter_context(tc.tile_pool(name="xa", bufs=3))
        q8a = a_stack.enter_context(tc.tile_pool(name="q8a", bufs=2))
        xqa = a_stack.enter_context(tc.tile_pool(name="xqa", bufs=2))
        sca = a_stack.enter_context(tc.tile_pool(name="sca", bufs=4))
        pta = a_stack.enter_context(tc.tile_pool(name="pta", bufs=2, space="PSUM"))

        # ---------------- Phase A: x quant + transpose -> xT (bf16) --------
        # processed in 32 quarter-row-tiles [P, 512] (4 k-blocks each)
        HB = 4                      # k-blocks per quarter-tile
        HN = HB * P                 # 512 columns
        for mi in range(MT):
            for h in range(4):
                xt = xa.tile([P, HN], F32)
                nc.sync.dma_start(out=xt[:], in_=x_in[:, mi, HN * h:HN * (h + 1)])
                amax = sca.tile([P, HB], F32, tag="amax")
                nc.vector.tensor_reduce(
                    amax[:], xt[:].rearrange("p (kb b) -> p kb b", b=P),
                    axis=mybir.AxisListType.X, op=MAXOP,
                    apply_absolute_value=True)
                nc.gpsimd.tensor_scalar_max(amax[:], amax[:], EPS)
                rcp = sca.tile([P, HB], F32, tag="rcp")
                nc.vector.reciprocal_approx_fast(out=rcp[:], in_=amax[:])
                inv2 = sca.tile([P, HB], F32, tag="inv2")
                nc.gpsimd.tensor_scalar_mul(inv2[:], rcp[:], 224.0)
                s2 = sca.tile([P, HB], F32, tag="s2")
                nc.gpsimd.tensor_scalar_mul(s2[:], amax[:], C224INV)
                q8 = q8a.tile([P, HN], FP8)
                xq = xqa.tile([P, HN], F16)
                for b in range(HB):
                    sl = slice(P * b, P * (b + 1))
                    # fp8 code: RNE(x * (224/amax)) via halved-scale TRN-e4m3
                    nc.scalar.activation(q8[:, sl], xt[:, sl], COPY,
                                         scale=inv2[:, b:b + 1])
                    # dequant: code * (amax/224) -> bf16 (split DVE/Act)
                    if b < 3:
                        nc.vector.tensor_scalar(
                            xq[:, sl], q8[:, sl], s2[:, b:b + 1], None,
                            op0=MULT)
                    else:
                        nc.scalar.activation(xq[:, sl], q8[:, sl], COPY,
                                             scale=s2[:, b:b + 1])
                pt = pta.tile([P, HB, P], F16)
                for b in range(HB):
                    nc.tensor.transpose(pt[:, b, :], xq[:, P * b:P * (b + 1)],
                                        ident[:])
                nc.vector.tensor_copy(
                    xT[:, HB * h:HB * (h + 1), P * mi:P * (mi + 1)], pt[:])
        a_stack.close()

        # ------- Phase B: GEMM1 + bias + gelu + h-quant + transpose -------
        # n-tiles 256 wide; m-halves so B(q=0) starts once A(mi<4) is done
        for q in range(2):
            for ni in range(NT1):
                w1t = w1p.tile([P, KB1, NB], F16)
                nc.sync.dma_start(
                    out=w1t[:], in_=w1_in[:, :, NB * ni:NB * (ni + 1)])
                b1t = b1p.tile([P, NB], F32)
                nc.sync.dma_start(
                    out=b1t[:], in_=bass.AP(b1_in, NB * ni, [[0, P], [1, NB]]))
                for mi in range(4 * q, 4 * q + 4):
                    ps = pp.tile([P, NB], F32)
                    for kb in range(KB1):
                        nc.tensor.matmul(
                            ps[:], xT[:, kb, P * mi:P * (mi + 1)], w1t[:, kb, :],
                            start=(kb == 0), stop=(kb == KB1 - 1))
                    z = zp.tile([P, NB], F32)
                    nc.vector.tensor_tensor(z[:], ps[:], b1t[:], op=ADD)
                    g = gp.tile([P, NB], F32)
                    nc.scalar.activation(g[:], z[:], GELU)
                    amaxh = scb.tile([P, 2], F32, tag="amaxh")
                    nc.vector.tensor_reduce(
                        amaxh[:], g[:].rearrange("p (nb b) -> p nb b", b=P),
                        axis=mybir.AxisListType.X, op=MAXOP,
                        apply_absolute_value=True)
                    nc.vector.tensor_scalar_max(amaxh[:], amaxh[:], EPS)
                    rch = scb.tile([P, 2], F32, tag="rch")
                    nc.vector.reciprocal_approx_fast(out=rch[:], in_=amaxh[:])
                    inv2h = scb.tile([P, 2], F32, tag="inv2h")
                    nc.vector.tensor_scalar_mul(inv2h[:], rch[:], 224.0)
                    s2h = scb.tile([P, 2], F32, tag="s2h")
                    nc.vector.tensor_scalar_mul(s2h[:], amaxh[:], C224INV)
                    h8 = h8p.tile([P, NB], FP8)
                    hq = hqp.tile([P, NB], F16)
                    for b in range(2):
                        sl = slice(P * b, P * (b + 1))
                        nc.scalar.activation(h8[:, sl], g[:, sl], COPY,
                                             scale=inv2h[:, b:b + 1])
                        nc.scalar.activation(hq[:, sl], h8[:, sl], COPY,
                                             scale=s2h[:, b:b + 1])
                    if mi % 2 == 0:
                        pt = ptb.tile([P, 2, 2, P], F16, tag="pt")
                        pt_pair = pt
                    for b in range(2):
                        nc.tensor.transpose(pt_pair[:, b, mi % 2, :],
                                            hq[:, P * b:P * (b + 1)], ident[:])
                    if mi % 2 == 1:
                        nc.vector.tensor_copy(
                            hT[:, 2 * ni:2 * ni + 2, P * (mi - 1):P * (mi + 1)],
                            pt_pair[:])
        b_stack.close()

        # ---------------- Phase C: GEMM2 + bias2 ----------------
        with contextlib.ExitStack() as c_stack:
            b2p = c_stack.enter_context(tc.tile_pool(name="b2p", bufs=2))
            op_ = c_stack.enter_context(tc.tile_pool(name="op", bufs=3))
            pc = c_stack.enter_context(tc.tile_pool(name="pc", bufs=8,
                                                    space="PSUM"))
            for ji in range(JT):
                b2t = b2p.tile([P, J], F32)
                nc.sync.dma_start(
                    out=b2t[:], in_=bass.AP(b2_in, J * ji, [[0, P], [1, J]]))
                pss = [pc.tile([P, J], F32, name="pss", tag="pss")
                       for _ in range(MT)]
                for kc in range(NKC):
                    w2c = w2p.tile([P, KC, J], F16)
                    nc.sync.dma_start(
                        out=w2c[:],
                        in_=w2_in[:, KC * kc:KC * (kc + 1), J * ji:J * (ji + 1)])
                    for mi in range(MT):
                        for kb in range(KC):
                            nc.tensor.matmul(
                                pss[mi][:],
                                hT[:, KC * kc + kb, P * mi:P * (mi + 1)],
                                w2c[:, kb, :],
                                start=(kc == 0 and kb == 0),
                                stop=(kc == NKC - 1 and kb == KC - 1))
                        if kc == NKC - 1:
                            # drain interleaved with remaining mi matmuls
                            ot = op_.tile([P, J], F32)
                            nc.vector.tensor_tensor(
                                ot[:], pss[mi][:], b2t[:], op=ADD)
                            dmaeng = nc.scalar if mi % 2 else nc.sync
                            dmaeng.dma_start(
                                out=out[P * mi:P * (mi + 1),
                                        J * ji:J * (ji + 1)],
                                in_=ot[:])

    nc.compile()
    return nc


_NC = None
last_results = None


def _get_nc():
    global _NC
    if _NC is None:
        _NC = _build()
    return _NC


def _fq8_rows(w: np.ndarray) -> np.ndarray:
    """Reference fp8 row-blockwise fake-quant (bitwise-exact, OCP e4m3fn)."""
    K, N = w.shape
    wb = w.reshape(K // P, P, N)
    scale = (np.maximum(np.abs(wb).max(axis=1, keepdims=True), EPS)
             / np.float32(448.0)).astype(np.float32)
    q = (wb / scale).astype(ml_dtypes.float8_e4m3fn).astype(np.float32) * scale
    return q.reshape(K, N).astype(np.float32)


def _prepare_in_maps(x, kernel1, bias1, kernel2, bias2):
    x = np.ascontiguousarray(np.asarray(x, dtype=np.float32))
    k1 = np.asarray(kernel1, dtype=np.float32)
    k2 = np.asarray(kernel2, dtype=np.float32)
    b1 = np.ascontiguousarray(np.asarray(bias1, dtype=np.float32))
    b2 = np.ascontiguousarray(np.asarray(bias2, dtype=np.float32))

    # Host-side static weight fake-quant (+ packing to [P, K//P, N] bf16).
    w1q = _fq8_rows(k1)
    w2q = _fq8_rows(k2)
    w1p = np.ascontiguousarray(
        w1q.reshape(KB1, P, EXPERT).transpose(1, 0, 2).astype(np.float16))
    w2p = np.ascontiguousarray(
        w2q.reshape(KB2, P, EXPERT).transpose(1, 0, 2).astype(np.float16))

    xf = x.reshape(ROWS, D_MODEL)
    in_maps = []
    for c in range(NCORES):
        xs = xf[MC * c:MC * (c + 1)]
        xp = np.ascontiguousarray(xs.reshape(MT, P, D_MODEL).transpose(1, 0, 2))
        in_maps.append({"xp": xp, "w1p": w1p, "b1": b1, "w2p": w2p, "b2": b2})
    return in_maps


def kernel(x, kernel1, bias1, kernel2, bias2):
    global last_results
    nc = _get_nc()
    in_maps = _prepare_in_maps(x, kernel1, bias1, kernel2, bias2)
    last_results = run_bass_kernel_spmd(nc, in_maps, core_ids=list(range(NCORES)))
    outs = [last_results.results[c]["out"] for c in range(NCORES)]
    full = np.concatenate(outs, axis=0).reshape(4, 2048, EXPERT)
    return full.astype(np.float32)
